# revision 3
# baseline (speedup 1.0000x reference)
"""GAT layer (nn_GAT_layer) Trainium2 Bass kernel — 8-core SPMD, row-sharded.

v3: hot loop = one DVE tensor_tensor per key tile, deep mask-DMA prefetch.

Hardware-tuned facts baked in (measured on these cores):
  - DVE ops pay a post-op DRAIN (~dur-266ns); per-tile DVE op count kept at 1.
  - GPSIMD elementwise is 5-20us per [128,1024] op: not used in the hot loop.
  - scalar_tensor_tensor is 1x on DVE (and invalid on Pool): the clamp
    (tensor_scalar max) is hoisted out of the loop entirely (t_all resident).
  - The mask stream must be prefetched several groups deep (PFD=6 groups of
    GRP=2 tiles) or the DVE starves on DMA jitter (~14us/rep penalty).

Math (per core c, query rows i in R_c, all keys j):
  weight_ij (up to a per-row factor that cancels in softmax)
    = m_ij * exp(0.2*s2_j) * exp(0.8*relu(x_ij)),   x_ij = s1_i + a_b + s2_j
  exp(0.8*relu(x)) = max(exp(0.8*x), 1) = e2_j * max(e1_i, cl2_j)
  with e1_i = exp(0.8*(s1_i+a_b)), cl2_j = exp(-0.8*s2_j).  Therefore
    weight_ij = [m_ij * t_ij] * G_j,
      t_ij  = max(e1_i, cl2_j)     (mask-independent!)
      G_j   = exp(s2_j - C2)       (folded into PE stationary [G*h_hat | G])
  t is precomputed for all 64 key tiles into SBUF (128 KiB/partition), so the
  per-tile steady-state work is ONE f16 tensor_tensor y = t * m.  That op is
  split into TT_SPLIT sub-ops of free-dim <= ~390 so each stays under the
  DVE post-op DRAIN knee (drain ~= max(0, dur-266ns)).
  Row normalization via the G-column; epilogue normalizes + ELU as v1.

Mask arrives as uint8 [N keys, RPC queries] (host pre-transposed), streamed
via SWDGE cast-DMA (u8 -> f16) in groups of GRP key tiles.

Self-contained: hardcodes shapes from the problem spec; no sibling imports.
"""

import os
import sys

import numpy as np

for _p in ("/opt/trn_rl_repo", "/root/.axon_site/_ro/trn_rl_repo"):
    if os.path.isdir(_p) and _p not in sys.path:
        sys.path.insert(0, _p)

import concourse.bass as bass
import concourse.bacc as bacc
import concourse.tile as tile
from concourse import mybir
from concourse.masks import make_identity
from concourse.bass_utils import run_bass_kernel_spmd

N, FIN, FOUT, CORES = 8192, 256, 64, 8
P = 128
RPC = N // CORES            # 1024 query rows per core
NJT = N // P                # 64 key tiles (j on partitions)
NHT = N // P                # 64 h row-tiles
NIB = RPC // P              # 8 output row-blocks per core
KC = FIN // P               # 2 contraction chunks for h_hat
HALF = RPC // 2             # 512: one PSUM bank of f32 per partition
C2 = 0.3                    # stationary shift (cancels in softmax)
E1CAP = 30000.0             # f16-safety caps (ranges give big margins)

GRP = 2                     # key tiles per mask DMA group
PFD = 6                     # mask DMA prefetch depth (groups in flight)
M16BUFS = 8                 # mask pool buffers (>= PFD + 1 ideally)
NGPS = 0                    # tiles per group handed to GPSIMD (u8 mask)
NGPS16 = 0                  # tiles per group handed to GPSIMD (f16 mask)
TT_SPLIT = 1                # sub-ops per DVE tensor_tensor

f32 = mybir.dt.float32
f16 = mybir.dt.float16
u8 = mybir.dt.uint8
AF = mybir.ActivationFunctionType
OP = mybir.AluOpType


def build_nc(reps: int = 1, dyn_reps: int = 0,
             grp: int = GRP, ngps: int = NGPS, ngps16: int = NGPS16,
             tt_split: int = TT_SPLIT,
             segs: tuple | None = None,
             pfd: int = PFD, m16bufs: int = M16BUFS) -> bass.Bass:
    nc = bacc.Bacc(None)

    h_full = nc.dram_tensor("h_full", [N, FIN], f32, kind="ExternalInput")[:]
    h_rows = nc.dram_tensor("h_rows", [RPC, FIN], f32, kind="ExternalInput")[:]
    mask_d = nc.dram_tensor("maskT_u8", [N, RPC], u8, kind="ExternalInput")[:]
    w_w = nc.dram_tensor("W_w", [FOUT, FIN], f32, kind="ExternalInput")[:]
    w_b = nc.dram_tensor("W_b_row", [1, FOUT], f32, kind="ExternalInput")[:]
    a1_d = nc.dram_tensor("a1_col", [FOUT, 1], f32, kind="ExternalInput")[:]
    a2_d = nc.dram_tensor("a2_row", [1, FOUT], f32, kind="ExternalInput")[:]
    ab_d = nc.dram_tensor("a_b_s", [1, 1], f32, kind="ExternalInput")[:]
    out_d = nc.dram_tensor("out_rows", [RPC, FOUT], f32, kind="ExternalOutput")[:]

    n16 = grp - ngps            # f16 cast-DMA tiles per group (DVE)
    # free-dim segment bounds for the split TT (even element counts)
    if segs is not None:
        seg = list(zip(segs[:-1], segs[1:]))
    else:
        seg = []
        s0 = 0
        for k in range(tt_split):
            s1_ = ((RPC * (k + 1)) // tt_split + 1) & ~1
            s1_ = min(s1_, RPC)
            seg.append((s0, s1_ if k < tt_split - 1 else RPC))
            s0 = seg[-1][1]

    with tile.TileContext(nc) as tc:
        with tc.tile_pool(name="consts", bufs=1) as consts:
            ident = consts.tile([P, P], f32)
            make_identity(nc, ident)
            ident16 = consts.tile([P, P], f16)
            make_identity(nc, ident16)
            ones1 = consts.tile([1, P], f32)
            nc.vector.memset(ones1, 1.0)

            ww_sb = consts.tile([FOUT, FIN], f16)
            nc.gpsimd.dma_start(out=ww_sb, in_=w_w)
            wb_sb = consts.tile([1, FOUT], f32)
            nc.gpsimd.dma_start(out=wb_sb, in_=w_b)
            wb_col = consts.tile([FOUT, 1], f32)
            nc.gpsimd.dma_start(out=wb_col, in_=w_b.rearrange("o f -> f o"))
            a1_sb = consts.tile([FOUT, 1], f32)
            nc.gpsimd.dma_start(out=a1_sb, in_=a1_d)
            a2_sb = consts.tile([1, FOUT], f32)
            nc.gpsimd.dma_start(out=a2_sb, in_=a2_d)
            ab_sb = consts.tile([1, 1], f32)
            nc.gpsimd.dma_start(out=ab_sb, in_=ab_d)

            wwt_sb = consts.tile([P, KC * FOUT], f16)     # W_w^T chunks
            hh2 = consts.tile([P, NJT, FOUT + 1], f16)    # [G*h_hat | G]
            t_all = consts.tile([P, NJT, RPC], f16)       # max(e1_i, cl2_j)
            cl2a = consts.tile([P, NJT], f32)
            e1b = consts.tile([P, RPC], f16)

            with tc.tile_pool(name="setup", bufs=1) as setup:
                # --- broadcast helpers
                wb_rep = setup.tile([1, 8 * FOUT], f32)
                for g in range(8):
                    nc.scalar.copy(wb_rep[:, g * FOUT:(g + 1) * FOUT], wb_sb)

                a2b_sb = setup.tile([P, FOUT], f16)
                wb_bc = setup.tile([P, 8 * FOUT], f32)
                with tc.tile_pool(name="ps_init", bufs=2,
                                  space="PSUM") as ps_init:
                    ps_w = ps_init.tile([P, KC * FOUT], f16, tag="w")
                    for kc in range(KC):
                        nc.tensor.transpose(
                            ps_w[:, kc * FOUT:(kc + 1) * FOUT],
                            ww_sb[:, kc * P:(kc + 1) * P],
                            ident16[0:FOUT, 0:FOUT],
                        )
                    nc.vector.tensor_copy(wwt_sb, ps_w)

                    ps_a2 = ps_init.tile([P, FOUT], f32, tag="a2")
                    nc.tensor.matmul(ps_a2, lhsT=ones1, rhs=a2_sb, start=True,
                                     stop=True)
                    nc.vector.tensor_copy(a2b_sb, ps_a2)

                    ps_wb = ps_init.tile([P, 8 * FOUT], f32, tag="wb")
                    nc.tensor.matmul(ps_wb, lhsT=ones1, rhs=wb_rep, start=True,
                                     stop=True)
                    nc.vector.tensor_copy(wb_bc, ps_wb)

                # --- h_hat for all N nodes (j on partitions), f16
                hh_raw = setup.tile([P, NJT, FOUT], f16)
                with (
                    tc.tile_pool(name="hload", bufs=3) as hload,
                    tc.tile_pool(name="hT", bufs=3) as h_t_pool,
                    tc.tile_pool(name="ps_T", bufs=2, space="PSUM") as ps_t_pool,
                    tc.tile_pool(name="ps_hh", bufs=2, space="PSUM") as ps_hh_pool,
                ):
                    ps_hh = None
                    for ht in range(NHT):
                        h_t = hload.tile([P, FIN], f16, tag="h")
                        nc.gpsimd.dma_start(
                            out=h_t, in_=h_full[ht * P:(ht + 1) * P, :])
                        ps_ht = ps_t_pool.tile([P, FIN], f16, tag="t")
                        for kc in range(KC):
                            nc.tensor.transpose(
                                ps_ht[:, kc * P:(kc + 1) * P],
                                h_t[:, kc * P:(kc + 1) * P],
                                ident16,
                            )
                        ht_sb = h_t_pool.tile([P, FIN], f16, tag="ht")
                        if ht % 2 == 0:
                            nc.vector.tensor_copy(ht_sb, ps_ht)
                        else:
                            nc.scalar.copy(ht_sb, ps_ht)

                        slot = ht % 8
                        if slot == 0:
                            ps_hh = ps_hh_pool.tile([P, 8 * FOUT], f32, tag="hh")
                        for kc in range(KC):
                            nc.tensor.matmul(
                                ps_hh[:, slot * FOUT:(slot + 1) * FOUT],
                                lhsT=ht_sb[:, kc * P:(kc + 1) * P],
                                rhs=wwt_sb[:, kc * FOUT:(kc + 1) * FOUT],
                                start=(kc == 0),
                                stop=(kc == KC - 1),
                            )
                        if slot == 7:
                            g = ht // 8
                            nc.vector.tensor_tensor(
                                out=hh_raw[:, g * 8:(g + 1) * 8, :],
                                in0=ps_hh[:].rearrange("p (a b) -> p a b", b=FOUT),
                                in1=wb_bc[:].rearrange("p (a b) -> p a b", b=FOUT),
                                op=OP.add,
                            )

                # --- s2 per key -> cl2 (clamp) and G (stationary scale)
                s2a = setup.tile([P, NJT], f32)
                sc = setup.tile([P, NJT, FOUT], f16)
                a2b_ap = a2b_sb[:]
                a2b_rep = bass.AP(
                    tensor=a2b_ap.tensor, offset=a2b_ap.offset,
                    ap=[list(a2b_ap.ap[0]), [0, NJT], list(a2b_ap.ap[1])],
                )
                nc.vector.tensor_tensor(out=sc, in0=hh_raw, in1=a2b_rep,
                                        op=OP.mult)
                nc.vector.tensor_reduce(
                    out=s2a[:].rearrange("p (a o) -> p a o", o=1), in_=sc,
                    axis=mybir.AxisListType.X, op=OP.add,
                )
                cl2r = setup.tile([P, NJT], f32)
                nc.scalar.activation(out=cl2r, in_=s2a, func=AF.Exp, scale=-0.8)
                nc.vector.tensor_scalar_min(cl2a, cl2r, E1CAP)
                ga = setup.tile([P, NJT], f32)
                negc2 = setup.tile([P, 1], f32)
                nc.vector.memset(negc2, -C2)
                nc.scalar.activation(out=ga, in_=s2a, func=AF.Exp, bias=negc2,
                                     scale=1.0)

                # --- stationary hh2 = [G*h_hat | G] per key tile, f16
                for t in range(NJT):
                    nc.vector.tensor_scalar(
                        hh2[:, t, 0:FOUT], hh_raw[:, t, :], ga[:, t:t + 1],
                        None, OP.mult,
                    )
                    nc.scalar.copy(hh2[:, t, FOUT:FOUT + 1], ga[:, t:t + 1])

                # --- s1 for this core's rows -> e1 (broadcast, f16)
                hhatt_sb = setup.tile([FOUT, RPC], f32)
                with (
                    tc.tile_pool(name="hload2", bufs=2) as hload2,
                    tc.tile_pool(name="hT2", bufs=2) as h_t2_pool,
                    tc.tile_pool(name="ps_T2", bufs=2, space="PSUM") as ps_t2_pool,
                    tc.tile_pool(name="ps_hhT", bufs=2, space="PSUM") as ps_hht_pool,
                    tc.tile_pool(name="ps_s1", bufs=1, space="PSUM") as ps_s1_pool,
                ):
                    ps_hht = None
                    for rt in range(NIB):
                        hr_t = hload2.tile([P, FIN], f16, tag="hr")
                        nc.gpsimd.dma_start(
                            out=hr_t, in_=h_rows[rt * P:(rt + 1) * P, :])
                        ps_htr = ps_t2_pool.tile([P, FIN], f16, tag="t2")
                        for kc in range(KC):
                            nc.tensor.transpose(
                                ps_htr[:, kc * P:(kc + 1) * P],
                                hr_t[:, kc * P:(kc + 1) * P],
                                ident16,
                            )
                        htr_sb = h_t2_pool.tile([P, FIN], f16, tag="htr")
                        nc.vector.tensor_copy(htr_sb, ps_htr)

                        slot = rt % 4
                        if slot == 0:
                            ps_hht = ps_hht_pool.tile([FOUT, 4 * P], f32,
                                                      tag="hht")
                        for kc in range(KC):
                            nc.tensor.matmul(
                                ps_hht[:, slot * P:(slot + 1) * P],
                                lhsT=wwt_sb[:, kc * FOUT:(kc + 1) * FOUT],
                                rhs=htr_sb[:, kc * P:(kc + 1) * P],
                                start=(kc == 0),
                                stop=(kc == KC - 1),
                            )
                        if slot == 3:
                            g = rt // 4
                            nc.scalar.activation(
                                out=hhatt_sb[:, g * 4 * P:(g + 1) * 4 * P],
                                in_=ps_hht,
                                func=AF.Identity,
                                bias=wb_col,
                                scale=1.0,
                            )

                    ps_s1 = ps_s1_pool.tile([1, RPC], f32, tag="s1")
                    for hf in range(2):
                        nc.tensor.matmul(
                            ps_s1[:, hf * HALF:(hf + 1) * HALF],
                            lhsT=a1_sb,
                            rhs=hhatt_sb[:, hf * HALF:(hf + 1) * HALF],
                            start=True,
                            stop=True,
                        )
                    s1row = setup.tile([1, RPC], f32)
                    nc.vector.tensor_scalar(s1row, ps_s1, ab_sb, None, OP.add)
                    e1r = setup.tile([1, RPC], f32)
                    nc.scalar.activation(out=e1r, in_=s1row, func=AF.Exp,
                                         scale=0.8)
                    e1c = setup.tile([1, RPC], f32)
                    nc.vector.tensor_scalar_min(e1c, e1r, E1CAP)

                    ps_e1b = ps_s1_pool.tile([P, RPC], f32, tag="e1b")
                    for hf in range(2):
                        nc.tensor.matmul(
                            ps_e1b[:, hf * HALF:(hf + 1) * HALF],
                            lhsT=ones1,
                            rhs=e1c[:, hf * HALF:(hf + 1) * HALF],
                            start=True,
                            stop=True,
                        )
                    nc.vector.tensor_copy(e1b, ps_e1b)

                # --- hoisted t_all[j-tile] = max(e1_i, cl2_j), f16
                for t in range(NJT):
                    nc.vector.tensor_scalar(
                        t_all[:, t, :], e1b, cl2a[:, t:t + 1], None, OP.max,
                    )
            # setup pools closed; SBUF freed for the mask stream

            # ---- main loop over key tiles: y = t * m, accumulate res^T
            NGRP = NJT // grp
            with (
                tc.tile_pool(name="m16p", bufs=m16bufs) as m16p,
                tc.tile_pool(name="m8p", bufs=2) as m8p,
                tc.tile_pool(name="yp", bufs=4) as yp,
                tc.tile_pool(name="ps_res", bufs=1, space="PSUM") as ps_res_pool,
                tc.tile_pool(name="ps_epi", bufs=2, space="PSUM") as ps_epi_pool,
                tc.tile_pool(name="epi", bufs=1) as epi,
                tc.tile_pool(name="outp", bufs=1) as outp,
            ):
                res_ps = ps_res_pool.tile([FOUT + 1, RPC], f32)

                from contextlib import nullcontext

                def rep_ctx():
                    return (tc.For_i(0, dyn_reps, 1) if dyn_reps > 1
                            else nullcontext())

                def issue_group_dmas(gi):
                    base = gi * grp * P
                    m16_t = m8_t = None
                    if n16:
                        m16_t = m16p.tile([P, n16, RPC], f16, tag="m16")
                        nc.gpsimd.dma_start(
                            out=m16_t,
                            in_=mask_d[base:base + n16 * P, :].rearrange(
                                "(t p) i -> p t i", p=P),
                        )
                    if ngps:
                        m8_t = m8p.tile([P, ngps, RPC], u8, tag="m8")
                        nc.sync.dma_start(
                            out=m8_t,
                            in_=mask_d[base + n16 * P:base + grp * P, :]
                            .rearrange("(t p) i -> p t i", p=P),
                        )
                    return m16_t, m8_t

                with rep_ctx():
                  for rep in range(reps):
                    issued = {g: issue_group_dmas(g) for g in range(min(pfd, NGRP))}
                    for jt in range(NJT):
                        gi, pos = divmod(jt, grp)
                        if pos == 0 and gi + pfd < NGRP:
                            issued[gi + pfd] = issue_group_dmas(gi + pfd)
                        m16_t, m8_t = issued[gi]
                        y_t = yp.tile([P, RPC], f16, tag="y")
                        if pos < n16 - ngps16:
                            for (a, b) in seg:
                                nc.vector.tensor_tensor(
                                    out=y_t[:, a:b],
                                    in0=t_all[:, jt, a:b],
                                    in1=m16_t[:, pos, a:b],
                                    op=OP.mult,
                                )
                        elif pos < n16:
                            nc.gpsimd.tensor_tensor(
                                out=y_t,
                                in0=t_all[:, jt, :],
                                in1=m16_t[:, pos, :],
                                op=OP.mult,
                            )
                        else:
                            nc.gpsimd.tensor_tensor(
                                out=y_t,
                                in0=t_all[:, jt, :],
                                in1=m8_t[:, pos - n16, :],
                                op=OP.mult,
                            )
                        for hf in range(2):
                            nc.tensor.matmul(
                                res_ps[:, hf * HALF:(hf + 1) * HALF],
                                lhsT=hh2[:, jt, :],
                                rhs=y_t[:, hf * HALF:(hf + 1) * HALF],
                                start=(jt == 0),
                                stop=(jt == NJT - 1),
                            )

                    # ---- epilogue: transpose res^T back, normalize, ELU, store
                    res_sb = epi.tile([FOUT + 1, RPC], f32, tag="res")
                    nc.scalar.copy(res_sb, res_ps)
                    # 8 transposed blocks packed into one PSUM tile, padded to
                    # 128 f32 per block so no block crosses a PSUM bank
                    ps8 = ps_epi_pool.tile([P, NIB, P], f32, tag="pst")
                    for ib in range(NIB):
                        nc.tensor.transpose(
                            ps8[:, ib, 0:FOUT + 1],
                            res_sb[:, ib * P:(ib + 1) * P],
                            ident[0:FOUT + 1, 0:FOUT + 1],
                        )
                    cp8 = epi.tile([P, NIB, FOUT + 1], f32, tag="cp8")
                    nc.scalar.copy(cp8, ps8[:, :, 0:FOUT + 1])
                    r8 = epi.tile([P, NIB, 1], f32, tag="recip")
                    nc.vector.reciprocal(r8, cp8[:, :, FOUT:FOUT + 1])
                    r8_ap = r8[:]
                    r8_rep = bass.AP(
                        tensor=r8_ap.tensor, offset=r8_ap.offset,
                        ap=[list(r8_ap.ap[0]), list(r8_ap.ap[1]), [0, FOUT]],
                    )
                    o8 = epi.tile([P, NIB, FOUT], f32, tag="o")
                    nc.vector.tensor_tensor(
                        out=o8, in0=cp8[:, :, 0:FOUT], in1=r8_rep, op=OP.mult)
                    xm = epi.tile([P, NIB, FOUT], f32, tag="xm")
                    nc.vector.tensor_scalar_min(xm, o8, 0.0)
                    eu = epi.tile([P, NIB, FOUT], f32, tag="eu")
                    nc.scalar.activation(out=eu, in_=xm, func=AF.Exp)
                    fin = outp.tile([P, NIB, FOUT], f32, tag="fin")
                    nc.vector.scalar_tensor_tensor(
                        out=fin, in0=eu, scalar=-1.0, in1=o8,
                        op0=OP.add, op1=OP.max,
                    )
                    nc.sync.dma_start(
                        out=out_d[:].rearrange("(t p) f -> p t f", p=P),
                        in_=fin,
                    )
    nc.finalize()
    return nc


_NC_CACHE: dict[int, bass.Bass] = {}


def _get_nc(reps: int = 1) -> bass.Bass:
    if reps not in _NC_CACHE:
        _NC_CACHE[reps] = build_nc(reps)
    return _NC_CACHE[reps]


def make_in_maps(h, attn_mask, W_w, W_b, a_w, a_b):
    h = np.ascontiguousarray(np.asarray(h, dtype=np.float32))
    attn_mask = np.asarray(attn_mask)
    W_w = np.ascontiguousarray(np.asarray(W_w, dtype=np.float32))
    W_b = np.ascontiguousarray(np.asarray(W_b, dtype=np.float32))
    a_w = np.ascontiguousarray(np.asarray(a_w, dtype=np.float32))
    a_b = np.ascontiguousarray(np.asarray(a_b, dtype=np.float32))

    mask_T = attn_mask.T                     # [N keys, N queries] view
    wb_row = W_b.reshape(1, FOUT)
    a1_col = np.ascontiguousarray(a_w[0, :FOUT].reshape(FOUT, 1))
    a2_row = np.ascontiguousarray(a_w[:, FOUT:])
    ab_s = a_b.reshape(1, 1)

    in_maps = []
    for c in range(CORES):
        rows = slice(c * RPC, (c + 1) * RPC)
        in_maps.append({
            "h_full": h,
            "h_rows": h[rows],
            "maskT_u8": np.ascontiguousarray(mask_T[:, rows]).astype(np.uint8),
            "W_w": W_w,
            "W_b_row": wb_row,
            "a1_col": a1_col,
            "a2_row": a2_row,
            "a_b_s": ab_s,
        })
    return in_maps


def kernel(h, attn_mask, W_w, W_b, a_w, a_b):
    nc = _get_nc()
    in_maps = make_in_maps(h, attn_mask, W_w, W_b, a_w, a_b)
    results = run_bass_kernel_spmd(nc, in_maps, list(range(CORES))).results
    out = np.concatenate([r["out_rows"] for r in results], axis=0)
    return out.astype(np.float32)


if __name__ == "__main__":
    nc = build_nc()
    print("built OK; instructions:",
          sum(len(bb.instructions) for bb in nc.m.functions[0].blocks))
